# revision 31
# baseline (speedup 1.0000x reference)
"""AdvisorCrossAttentionAdapter Trainium2 kernel.

Full inputs in, full outputs out. Sharding: 8 cores = 4 batches x 2 query
halves. Each core computes its batch's V projection + id-gating (duplicated
across the 2 cores sharing a batch) and attention + output projection for its
1024-row query slice.

Math notes:
  - Wk is folded away on the host: scores = hidden @ (Wq.T @ Wk) @ adv0.T,
    with M = Wq.T @ Wk precomputed once per call. This deletes the whole
    K-projection phase.
  - The id-gate is rewritten as a linear part plus two sparse abs-terms:
      v_final = [c0*a0 + k_s*asum + k_d*adif] @ Wv.T
              + P_impl @ |(k_as*asum)_impl @ Wv.T|
              + P_aox~ @ |(|k_ad|*adif)_aox @ Wv.T|
    where the _impl rows (id==3, ~T/9) and _aox rows (id in {0,1,4}, ~T/3)
    are gathered compact on the host (padded to 256 / 512), and P_impl /
    P_aox~ are 0/±1 scatter matrices applied as matmuls accumulating into
    the same PSUM group as the linear part. All per-row coefficients are
    folded host-side; the device does no per-element gate arithmetic.
  - Softmax runs without max subtraction (scores/sqrt(h) ~ N(0,1), |max|
    < ~7, comfortably inside fp32 exp range); exp'd scores stay unnormalized
    through the ctx matmul and the 1/sum factor is applied on the ctx^T copy.
  - All matmuls take bf16 inputs with fp32 PSUM accumulation.
"""

import numpy as np
import ml_dtypes
from contextlib import ExitStack

P = 128
H = 2048          # hidden dim
HC = H // P       # 16 h-chunks of 128
T = 1024          # triplets per batch (advisor len 3072 / 3)
TC = T // P       # 8 t-chunks
S = 1024          # query rows per core (2048 / 2)
B = 4
NCORES = 8
IPAD = 256        # padded compact rows for impl gate (id==3)
APAD = 384        # padded compact rows for and/or/xor gates (id in {0,1,4})
TP2 = True        # tensor-parallel V projection across core pairs
SCALE = 1.0 / float(np.sqrt(H))

bf16 = ml_dtypes.bfloat16

_compiled_nc = None


def _build_nc(s_rows=S, t_trip=T, h=H, ipad=IPAD, apad=APAD, tp2=False,
              n_dev=NCORES):
    import concourse.bass as bass
    import concourse.mybir as mybir
    import concourse.tile as tile
    from concourse import bacc

    hc = h // P
    tc_n = t_trip // P
    sc_n = s_rows // P
    n512 = h // 512          # number of 512-wide o tiles
    s512 = s_rows // 512     # number of 512-wide s tiles
    ipc = ipad // P
    apc = apad // P
    # With tensor-parallel pairs, each core computes only its half of the V
    # projection's output columns (which half is decided purely by the wv
    # data the host feeds it) and an AllGather completes vf.
    hv = h // 2 if tp2 else h
    assert s_rows % 512 == 0 and h % 512 == 0 and hv % 512 == 0

    f32 = mybir.dt.float32
    bf = mybir.dt.bfloat16

    nc = bacc.Bacc("TRN2", target_bir_lowering=False, debug=False,
                   num_devices=n_dev)

    # DRAM I/O. Activations/weights are pre-transposed on the host into
    # [partition, chunk, cols] layout: x[p, c, n] = X[c*128+p, n] where X is
    # the [rows, n] transposed matrix.
    d_h = nc.dram_tensor("hT", [P, hc, s_rows], bf, kind="ExternalInput")
    d_a0 = nc.dram_tensor("a0", [P, hc, t_trip], bf, kind="ExternalInput")
    d_al = nc.dram_tensor("alT", [P, hc, t_trip], bf, kind="ExternalInput")
    d_si = nc.dram_tensor("siT", [P, hc, ipad], bf, kind="ExternalInput")
    d_da = nc.dram_tensor("daT", [P, hc, apad], bf, kind="ExternalInput")
    d_pi = nc.dram_tensor("piT", [P, ipc, t_trip], bf, kind="ExternalInput")
    d_pa = nc.dram_tensor("paT", [P, apc, t_trip], bf, kind="ExternalInput")
    d_m = nc.dram_tensor("m", [P, hc, h], bf, kind="ExternalInput")
    d_wv = nc.dram_tensor("wv", [P, hc, hv], bf, kind="ExternalInput")
    d_wo = nc.dram_tensor("wo", [P, hc, h], bf, kind="ExternalInput")
    d_out = nc.dram_tensor("out", [s_rows, h], f32, kind="ExternalOutput")

    AL = mybir.AluOpType
    AF = mybir.ActivationFunctionType

    with tile.TileContext(nc) as tc, ExitStack() as ctx:
        big = ctx.enter_context(tc.tile_pool(name="big", bufs=1))
        pw = ctx.enter_context(tc.tile_pool(name="pw", bufs=2))
        pws = ctx.enter_context(tc.tile_pool(name="pws", bufs=3))
        psd = ctx.enter_context(tc.tile_pool(name="psd", bufs=4))
        pgs = ctx.enter_context(tc.tile_pool(name="pgs", bufs=3))
        psm = ctx.enter_context(tc.tile_pool(name="psm", bufs=1))
        pp = ctx.enter_context(tc.tile_pool(name="pp", bufs=6, space="PSUM"))
        ppe = ctx.enter_context(tc.tile_pool(name="ppe", bufs=2, space="PSUM"))

        dram = None
        if tp2:
            dram = ctx.enter_context(tc.tile_pool(name="dram", bufs=1,
                                                  space="DRAM"))

        # Persistent intermediates
        a0 = big.tile([P, hc, t_trip], bf, tag="A", name="a0sb")
        vf = big.tile([P, tc_n, h], bf, tag="B", name="vf")
        absi = big.tile([P, ipc, hv], bf, tag="E", name="absi")
        absa = big.tile([P, apc, hv], bf, tag="D", name="absa")
        pi_sb = psm.tile([P, ipc, t_trip], bf, tag="pi", name="pi_sb")
        pa_sb = psm.tile([P, apc, t_trip], bf, tag="pa", name="pa_sb")

        nc.sync.dma_start(a0[:], d_a0[:])
        nc.sync.dma_start(pi_sb[:], d_pi[:])
        nc.sync.dma_start(pa_sb[:], d_pa[:])

        # ACT-written zero bias vector so Abs/Exp activations don't pull in a
        # DMA'd const AP. x*0 == 0 for finite x; also absorbs the pipeline-RAW
        # wait (3-source ISA formats only have one sync-wait slot).
        zbias = psm.tile([P, 1], f32, tag="zb", name="zbias")
        nc.scalar.mul(zbias[:], a0[:, 0, 0:1], 0.0)
        warm = psm.tile([P, 1], f32, tag="wm", name="warm")
        nc.scalar.copy(warm[:], zbias[:])

        # ------------- Phase V: gate-compact V projection -> vf -------------
        if tp2:
            vhalf_in = dram.tile([t_trip, hv], bf, name="vhalf_in")
            vhalf_out = dram.tile([2, t_trip, hv], bf, addr_space="Shared",
                                  name="vhalf_out")
        for ot in range(hv // 512):
            osl = slice(ot * 512, (ot + 1) * 512)
            wv_ot = pw.tile([P, hc, 512], bf, tag="W4", name="wv_ot")
            nc.sync.dma_start(wv_ot[:], d_wv[:, :, osl])
            # compact projections + abs
            for cc in range(ipc):
                si_cc = psd.tile([P, hc, P], bf, tag="SD", name="si_cc")
                nc.sync.dma_start(si_cc[:], d_si[:, :, cc * P:(cc + 1) * P])
                ps_i = pp.tile([P, 512], f32, tag="PS", name="ps_i")
                for ch in range(hc):
                    nc.tensor.matmul(ps_i[:], si_cc[:, ch, :], wv_ot[:, ch, :],
                                     start=(ch == 0), stop=(ch == hc - 1))
                nc.scalar.activation(absi[:, cc, osl], ps_i[:], AF.Abs,
                                     bias=zbias[:])
            for cc in range(apc):
                da_cc = psd.tile([P, hc, P], bf, tag="SD", name="da_cc")
                nc.sync.dma_start(da_cc[:], d_da[:, :, cc * P:(cc + 1) * P])
                ps_a = pp.tile([P, 512], f32, tag="PS", name="ps_a")
                for ch in range(hc):
                    nc.tensor.matmul(ps_a[:], da_cc[:, ch, :], wv_ot[:, ch, :],
                                     start=(ch == 0), stop=(ch == hc - 1))
                nc.scalar.activation(absa[:, cc, osl], ps_a[:], AF.Abs,
                                     bias=zbias[:])
            # linear part + scatter, accumulated in one PSUM group
            for tch in range(tc_n):
                tsl = slice(tch * P, (tch + 1) * P)
                al_t = psd.tile([P, hc, P], bf, tag="SD", name="al_t")
                nc.sync.dma_start(al_t[:], d_al[:, :, tsl])
                ps_v = pp.tile([P, 512], f32, tag="PS", name="ps_v")
                for ch in range(hc):
                    nc.tensor.matmul(ps_v[:], al_t[:, ch, :], wv_ot[:, ch, :],
                                     start=(ch == 0), stop=False)
                for cc in range(ipc):
                    nc.tensor.matmul(ps_v[:], pi_sb[:, cc, tsl],
                                     absi[:, cc, osl],
                                     start=False, stop=False)
                for cc in range(apc):
                    nc.tensor.matmul(ps_v[:], pa_sb[:, cc, tsl],
                                     absa[:, cc, osl],
                                     start=False, stop=(cc == apc - 1))
                if tp2:
                    vs = pgs.tile([P, 512], bf, tag="VS", name="vs")
                    nc.vector.tensor_copy(vs[:], ps_v[:])
                    nc.sync.dma_start(vhalf_in[tch * P:(tch + 1) * P, osl],
                                      vs[:])
                else:
                    nc.vector.tensor_copy(vf[:, tch, osl], ps_v[:])

        if tp2:
            nc.gpsimd.collective_compute(
                "AllGather",
                mybir.AluOpType.bypass,
                replica_groups=[[2 * i, 2 * i + 1] for i in range(n_dev // 2)],
                ins=[vhalf_in.opt()],
                outs=[vhalf_out.opt()],
            )
            for half in range(2):
                for tch in range(tc_n):
                    nc.sync.dma_start(
                        vf[:, tch, half * hv:(half + 1) * hv],
                        vhalf_out[half, tch * P:(tch + 1) * P, :])

        # ------------- Phase Q: qm^T[a, s] = M^T hidden^T -------------------
        qmT = big.tile([P, hc, s_rows], bf, tag="C", name="qmT")
        for sh in range(s512):
            hid = big.tile([P, hc, 512], bf, tag="D", name="hid")
            nc.sync.dma_start(hid[:], d_h[:, :, sh * 512:(sh + 1) * 512])
            for oc in range(hc):
                m_oc = pws.tile([P, hc, P], bf, tag="W1", name="m_oc")
                nc.sync.dma_start(m_oc[:], d_m[:, :, oc * P:(oc + 1) * P])
                ps_q = pp.tile([P, 512], f32, tag="PS", name="ps_q")
                for ch in range(hc):
                    nc.tensor.matmul(ps_q[:], m_oc[:, ch, :], hid[:, ch, :],
                                     start=(ch == 0), stop=(ch == hc - 1))
                nc.vector.tensor_copy(qmT[:, oc, sh * 512:(sh + 1) * 512],
                                      ps_q[:])

        # ------------- Phase S: scores^T, exp, sums -------------------------
        eT = big.tile([P, tc_n, s_rows], bf, tag="D", name="eT")
        ones_t = psm.tile([P, 1], bf, tag="o1", name="ones_t")
        nc.vector.memset(ones_t[:], 1.0)
        pse = []
        for st in range(s512):
            t_ = ppe.tile([P, 512], f32, tag="PSE", name="ps_sum")
            pse.append(t_)
        for tch in range(tc_n):
            ps_sc = []
            for st in range(s512):
                ps_x = pp.tile([P, 512], f32, tag="PS", name="ps_sc")
                ps_sc.append(ps_x)
                for ch in range(hc):
                    nc.tensor.matmul(ps_x[:], a0[:, ch, tch * P:(tch + 1) * P],
                                     qmT[:, ch, st * 512:(st + 1) * 512],
                                     start=(ch == 0), stop=(ch == hc - 1))
            for st in range(s512):
                nc.scalar.activation(eT[:, tch, st * 512:(st + 1) * 512],
                                     ps_sc[st][:], AF.Exp, bias=zbias[:],
                                     scale=SCALE)
            for st in range(s512):
                nc.tensor.matmul(pse[st][0:1, :], ones_t[:],
                                 eT[:, tch, st * 512:(st + 1) * 512],
                                 start=(tch == 0), stop=(tch == tc_n - 1))

        recip = psm.tile([1, s_rows], f32, tag="rc", name="recip")
        for st in range(s512):
            nc.vector.reciprocal(recip[:, st * 512:(st + 1) * 512],
                                 pse[st][0:1, :])
        # Broadcast partition 0 to all partitions via a K=1 fp32 matmul
        ones_b = psm.tile([1, P], f32, tag="ob1", name="ones_b")
        nc.vector.memset(ones_b[:], 1.0)
        bcast = psm.tile([P, s_rows], f32, tag="bc", name="bcast")
        for st in range(s512):
            ps_b = pp.tile([P, 512], f32, tag="PS", name="ps_b")
            nc.tensor.matmul(ps_b[:], ones_b[:],
                             recip[:, st * 512:(st + 1) * 512])
            nc.vector.tensor_copy(bcast[:, st * 512:(st + 1) * 512], ps_b[:])

        # ------------- Phase C: ctx^T[h, s] (normalized) --------------------
        cT = big.tile([P, hc, s_rows], bf, tag="A", name="cT")
        for ch in range(hc):
            for st in range(s512):
                ps_c = pp.tile([P, 512], f32, tag="PS", name="ps_c")
                for tch in range(tc_n):
                    nc.tensor.matmul(ps_c[:], vf[:, tch, ch * P:(ch + 1) * P],
                                     eT[:, tch, st * 512:(st + 1) * 512],
                                     start=(tch == 0), stop=(tch == tc_n - 1))
                nc.vector.tensor_tensor(cT[:, ch, st * 512:(st + 1) * 512],
                                        ps_c[:],
                                        bcast[:, st * 512:(st + 1) * 512],
                                        AL.mult)

        # ------------- Phase O: out[s, o] = ctx Wo^T ------------------------
        for ot in range(n512):
            osl = slice(ot * 512, (ot + 1) * 512)
            wo_ot = pw.tile([P, hc, 512], bf, tag="W4", name="wo_ot")
            nc.sync.dma_start(wo_ot[:], d_wo[:, :, osl])
            for sc in range(sc_n):
                ps_o = pp.tile([P, 512], f32, tag="PS", name="ps_o")
                for ch in range(hc):
                    nc.tensor.matmul(ps_o[:], cT[:, ch, sc * P:(sc + 1) * P],
                                     wo_ot[:, ch, :],
                                     start=(ch == 0), stop=(ch == hc - 1))
                ob = pgs.tile([P, 512], f32, tag="OB", name="ob")
                nc.vector.tensor_copy(ob[:], ps_o[:])
                nc.sync.dma_start(d_out[sc * P:(sc + 1) * P, osl], ob[:])

    nc.compile()
    return nc


def _to_dev_layout(x_t, rows):
    """[rows, n] fp32 -> [128, rows//128, n] bf16 contiguous."""
    rc = rows // P
    return np.ascontiguousarray(
        x_t.reshape(rc, P, -1).transpose(1, 0, 2).astype(bf16))


def _gate_prep(trip, rid, ipad, apad):
    """Host-side gate folding for one batch.

    trip: [T, 3, h] fp32; rid: [T] ids.
    Returns adv_lin [T,h], si [ipad,h], da [apad,h], Pi [T,ipad], Pa [T,apad].
    """
    t_n = trip.shape[0]
    m_and = rid == 0
    m_or = rid == 1
    m_not = rid == 2
    m_impl = rid == 3
    m_xor = rid == 4
    c0 = (rid >= 5).astype(np.float32)
    ca = m_and.astype(np.float32) - m_xor.astype(np.float32)
    cb = m_or.astype(np.float32) + m_xor.astype(np.float32)
    c1 = -(m_not.astype(np.float32))
    ci = m_impl.astype(np.float32)
    k_s = (ca + cb + c1) / 2
    k_d = (c1 - ci) / 2
    k_as = ci / 2
    k_ad = (cb - ca) / 2

    a0 = trip[:, 0]
    asum = trip[:, 1] + trip[:, 2]
    adif = trip[:, 1] - trip[:, 2]
    adv_lin = c0[:, None] * a0 + k_s[:, None] * asum + k_d[:, None] * adif

    h = trip.shape[2]
    impl_idx = np.where(m_impl)[0]
    aox_idx = np.where(m_and | m_or | m_xor)[0]
    assert len(impl_idx) <= ipad, f"impl rows {len(impl_idx)} > pad {ipad}"
    assert len(aox_idx) <= apad, f"aox rows {len(aox_idx)} > pad {apad}"
    si = np.zeros((ipad, h), np.float32)
    si[:len(impl_idx)] = k_as[impl_idx, None] * asum[impl_idx]
    da = np.zeros((apad, h), np.float32)
    da[:len(aox_idx)] = np.abs(k_ad[aox_idx, None]) * adif[aox_idx]
    Pi = np.zeros((t_n, ipad), np.float32)
    Pi[impl_idx, np.arange(len(impl_idx))] = 1.0
    Pa = np.zeros((t_n, apad), np.float32)
    Pa[aox_idx, np.arange(len(aox_idx))] = np.sign(k_ad[aox_idx])
    return adv_lin, si, da, Pi, Pa


def kernel(hidden_states, advisor_states, advisor_ids, Wq, Wk, Wv, Wo):
    from concourse.bass_utils import run_bass_kernel_spmd

    hs = np.asarray(hidden_states, dtype=np.float32)     # [4, 2048, 2048]
    adv = np.asarray(advisor_states, dtype=np.float32)   # [4, 3072, 2048]
    ids = np.asarray(advisor_ids)                        # [4, 3072]

    # Size the compact-gate pads to the data (multiple of 128, with the
    # compiled defaults as minimum). Rebuild only if the data needs more.
    rid_all = ids.reshape(B, T, 3)[:, :, 0]
    need_i = int(max((rid_all[b] == 3).sum() for b in range(B)))
    need_a = int(max(((rid_all[b] == 0) | (rid_all[b] == 1)
                      | (rid_all[b] == 4)).sum() for b in range(B)))
    ipad = max(IPAD, -(-need_i // P) * P)
    apad = max(APAD, -(-need_a // P) * P)

    global _compiled_nc
    if _compiled_nc is None or _compiled_nc[0] != (ipad, apad, TP2):
        _compiled_nc = ((ipad, apad, TP2),
                        _build_nc(ipad=ipad, apad=apad, tp2=TP2))
    nc = _compiled_nc[1]
    Wq = np.asarray(Wq, dtype=np.float32)
    Wk = np.asarray(Wk, dtype=np.float32)
    Wv = np.asarray(Wv, dtype=np.float32)
    Wo = np.asarray(Wo, dtype=np.float32)

    M = Wq.T @ Wk                                        # folds K projection
    w_dev = {
        "m": _to_dev_layout(np.ascontiguousarray(M), H),
        "wo": _to_dev_layout(np.ascontiguousarray(Wo.T), H),
    }
    WvT = Wv.T
    if TP2:
        wv_half = [
            _to_dev_layout(np.ascontiguousarray(WvT[:, :H // 2]), H),
            _to_dev_layout(np.ascontiguousarray(WvT[:, H // 2:]), H),
        ]
    else:
        wv_full = _to_dev_layout(np.ascontiguousarray(WvT), H)

    per_batch = []
    for b in range(B):
        trip = adv[b].reshape(T, 3, H)
        rid = ids[b].reshape(T, 3)[:, 0]
        adv_lin, si, da, Pi, Pa = _gate_prep(trip, rid, ipad, apad)
        per_batch.append({
            "a0": _to_dev_layout(np.ascontiguousarray(trip[:, 0].T), H),
            "alT": _to_dev_layout(np.ascontiguousarray(adv_lin.T), H),
            "siT": _to_dev_layout(np.ascontiguousarray(si.T), H),
            "daT": _to_dev_layout(np.ascontiguousarray(da.T), H),
            "piT": _to_dev_layout(np.ascontiguousarray(Pi.T), ipad),
            "paT": _to_dev_layout(np.ascontiguousarray(Pa.T), apad),
        })

    in_maps = []
    for c in range(NCORES):
        b, sh = c // 2, c % 2
        hT = np.ascontiguousarray(hs[b, sh * S:(sh + 1) * S, :].T)
        m = {
            "hT": _to_dev_layout(hT, H),
            "wv": wv_half[sh] if TP2 else wv_full,
            **per_batch[b],
            **w_dev,
        }
        in_maps.append(m)

    res = run_bass_kernel_spmd(nc, in_maps, core_ids=list(range(NCORES)))
    kernel._last_results = res

    out = np.empty((B, 2 * S, H), dtype=np.float32)
    for c in range(NCORES):
        b, sh = c // 2, c % 2
        out[b, sh * S:(sh + 1) * S, :] = res.results[c]["out"]
    return out


# revision 32
# speedup vs baseline: 1.0960x; 1.0960x over previous
"""AdvisorCrossAttentionAdapter Trainium2 kernel.

Full inputs in, full outputs out. Sharding: 8 cores = 4 batches x 2 query
halves. Each core computes its batch's V projection + id-gating (duplicated
across the 2 cores sharing a batch) and attention + output projection for its
1024-row query slice.

Math notes:
  - Wk is folded away on the host: scores = hidden @ (Wq.T @ Wk) @ adv0.T,
    with M = Wq.T @ Wk precomputed once per call. This deletes the whole
    K-projection phase.
  - The id-gate is rewritten as a linear part plus two sparse abs-terms:
      v_final = [c0*a0 + k_s*asum + k_d*adif] @ Wv.T
              + P_impl @ |(k_as*asum)_impl @ Wv.T|
              + P_aox~ @ |(|k_ad|*adif)_aox @ Wv.T|
    where the _impl rows (id==3, ~T/9) and _aox rows (id in {0,1,4}, ~T/3)
    are gathered compact on the host (padded to 256 / 512), and P_impl /
    P_aox~ are 0/±1 scatter matrices applied as matmuls accumulating into
    the same PSUM group as the linear part. All per-row coefficients are
    folded host-side; the device does no per-element gate arithmetic.
  - Softmax runs without max subtraction (scores/sqrt(h) ~ N(0,1), |max|
    < ~7, comfortably inside fp32 exp range); exp'd scores stay unnormalized
    through the ctx matmul and the 1/sum factor is applied on the ctx^T copy.
  - All matmuls take bf16 inputs with fp32 PSUM accumulation.
"""

import numpy as np
import ml_dtypes
from contextlib import ExitStack

P = 128
H = 2048          # hidden dim
HC = H // P       # 16 h-chunks of 128
T = 1024          # triplets per batch (advisor len 3072 / 3)
TC = T // P       # 8 t-chunks
S = 1024          # query rows per core (2048 / 2)
B = 4
NCORES = 8
IPAD = 256        # padded compact rows for impl gate (id==3)
APAD = 384        # padded compact rows for and/or/xor gates (id in {0,1,4})
TP2 = True        # tensor-parallel V projection across core pairs
SCALE = 1.0 / float(np.sqrt(H))

bf16 = ml_dtypes.bfloat16

_compiled_nc = None


def _build_nc(s_rows=S, t_trip=T, h=H, ipad=IPAD, apad=APAD, tp2=False,
              n_dev=NCORES):
    import concourse.bass as bass
    import concourse.mybir as mybir
    import concourse.tile as tile
    from concourse import bacc

    hc = h // P
    tc_n = t_trip // P
    sc_n = s_rows // P
    n512 = h // 512          # number of 512-wide o tiles
    s512 = s_rows // 512     # number of 512-wide s tiles
    ipc = ipad // P
    apc = apad // P
    # With tensor-parallel pairs, each core computes only its half of the V
    # projection's output columns (which half is decided purely by the wv
    # data the host feeds it) and an AllGather completes vf.
    hv = h // 2 if tp2 else h
    assert s_rows % 512 == 0 and h % 512 == 0 and hv % 512 == 0

    f32 = mybir.dt.float32
    bf = mybir.dt.bfloat16

    nc = bacc.Bacc("TRN2", target_bir_lowering=False, debug=False,
                   num_devices=n_dev)

    # DRAM I/O. Activations/weights are pre-transposed on the host into
    # [partition, chunk, cols] layout: x[p, c, n] = X[c*128+p, n] where X is
    # the [rows, n] transposed matrix.
    d_h = nc.dram_tensor("hT", [P, hc, s_rows], bf, kind="ExternalInput")
    d_a0 = nc.dram_tensor("a0", [P, hc, t_trip], bf, kind="ExternalInput")
    d_al = nc.dram_tensor("alT", [P, hc, t_trip], bf, kind="ExternalInput")
    d_si = nc.dram_tensor("siT", [P, hc, ipad], bf, kind="ExternalInput")
    d_da = nc.dram_tensor("daT", [P, hc, apad], bf, kind="ExternalInput")
    d_pi = nc.dram_tensor("piT", [P, ipc, t_trip], bf, kind="ExternalInput")
    d_pa = nc.dram_tensor("paT", [P, apc, t_trip], bf, kind="ExternalInput")
    d_m = nc.dram_tensor("m", [P, hc, h], bf, kind="ExternalInput")
    d_wv = nc.dram_tensor("wv", [P, hc, hv], bf, kind="ExternalInput")
    d_wo = nc.dram_tensor("wo", [P, hc, h], bf, kind="ExternalInput")
    d_out = nc.dram_tensor("out", [s_rows, h], f32, kind="ExternalOutput")

    AL = mybir.AluOpType
    AF = mybir.ActivationFunctionType

    with tile.TileContext(nc) as tc, ExitStack() as ctx:
        big = ctx.enter_context(tc.tile_pool(name="big", bufs=1))
        pw = ctx.enter_context(tc.tile_pool(name="pw", bufs=2))
        pws = ctx.enter_context(tc.tile_pool(name="pws", bufs=3))
        psd = ctx.enter_context(tc.tile_pool(name="psd", bufs=4))
        pgs = ctx.enter_context(tc.tile_pool(name="pgs", bufs=3))
        psm = ctx.enter_context(tc.tile_pool(name="psm", bufs=1))
        pp = ctx.enter_context(tc.tile_pool(name="pp", bufs=6, space="PSUM"))
        ppe = ctx.enter_context(tc.tile_pool(name="ppe", bufs=2, space="PSUM"))

        dram = None
        if tp2:
            dram = ctx.enter_context(tc.tile_pool(name="dram", bufs=1,
                                                  space="DRAM"))

        # Persistent intermediates
        a0 = big.tile([P, hc, t_trip], bf, tag="A", name="a0sb")
        vf = big.tile([P, tc_n, h], bf, tag="B", name="vf")
        absi = big.tile([P, ipc, hv], bf, tag="E", name="absi")
        absa = big.tile([P, apc, hv], bf, tag="D", name="absa")
        pi_sb = psm.tile([P, ipc, t_trip], bf, tag="pi", name="pi_sb")
        pa_sb = psm.tile([P, apc, t_trip], bf, tag="pa", name="pa_sb")

        nc.sync.dma_start(a0[:], d_a0[:])
        nc.sync.dma_start(pi_sb[:], d_pi[:])
        nc.sync.dma_start(pa_sb[:], d_pa[:])

        # ACT-written zero bias vector so Abs/Exp activations don't pull in a
        # DMA'd const AP. x*0 == 0 for finite x; also absorbs the pipeline-RAW
        # wait (3-source ISA formats only have one sync-wait slot).
        zbias = psm.tile([P, 1], f32, tag="zb", name="zbias")
        nc.scalar.mul(zbias[:], a0[:, 0, 0:1], 0.0)
        warm = psm.tile([P, 1], f32, tag="wm", name="warm")
        nc.scalar.copy(warm[:], zbias[:])

        # ------------- Phase V: gate-compact V projection -> vf -------------
        if tp2:
            vhalf_in = dram.tile([t_trip, hv], bf, name="vhalf_in")
            vhalf_out = dram.tile([2, t_trip, hv], bf, name="vhalf_out")
        for ot in range(hv // 512):
            osl = slice(ot * 512, (ot + 1) * 512)
            wv_ot = pw.tile([P, hc, 512], bf, tag="W4", name="wv_ot")
            nc.sync.dma_start(wv_ot[:], d_wv[:, :, osl])
            # compact projections + abs
            for cc in range(ipc):
                si_cc = psd.tile([P, hc, P], bf, tag="SD", name="si_cc")
                nc.sync.dma_start(si_cc[:], d_si[:, :, cc * P:(cc + 1) * P])
                ps_i = pp.tile([P, 512], f32, tag="PS", name="ps_i")
                for ch in range(hc):
                    nc.tensor.matmul(ps_i[:], si_cc[:, ch, :], wv_ot[:, ch, :],
                                     start=(ch == 0), stop=(ch == hc - 1))
                nc.scalar.activation(absi[:, cc, osl], ps_i[:], AF.Abs,
                                     bias=zbias[:])
            for cc in range(apc):
                da_cc = psd.tile([P, hc, P], bf, tag="SD", name="da_cc")
                nc.sync.dma_start(da_cc[:], d_da[:, :, cc * P:(cc + 1) * P])
                ps_a = pp.tile([P, 512], f32, tag="PS", name="ps_a")
                for ch in range(hc):
                    nc.tensor.matmul(ps_a[:], da_cc[:, ch, :], wv_ot[:, ch, :],
                                     start=(ch == 0), stop=(ch == hc - 1))
                nc.scalar.activation(absa[:, cc, osl], ps_a[:], AF.Abs,
                                     bias=zbias[:])
            # linear part + scatter, accumulated in one PSUM group
            for tch in range(tc_n):
                tsl = slice(tch * P, (tch + 1) * P)
                al_t = psd.tile([P, hc, P], bf, tag="SD", name="al_t")
                nc.sync.dma_start(al_t[:], d_al[:, :, tsl])
                ps_v = pp.tile([P, 512], f32, tag="PS", name="ps_v")
                for ch in range(hc):
                    nc.tensor.matmul(ps_v[:], al_t[:, ch, :], wv_ot[:, ch, :],
                                     start=(ch == 0), stop=False)
                for cc in range(ipc):
                    nc.tensor.matmul(ps_v[:], pi_sb[:, cc, tsl],
                                     absi[:, cc, osl],
                                     start=False, stop=False)
                for cc in range(apc):
                    nc.tensor.matmul(ps_v[:], pa_sb[:, cc, tsl],
                                     absa[:, cc, osl],
                                     start=False, stop=(cc == apc - 1))
                if tp2:
                    vs = pgs.tile([P, 512], bf, tag="VS", name="vs")
                    nc.vector.tensor_copy(vs[:], ps_v[:])
                    nc.sync.dma_start(vhalf_in[tch * P:(tch + 1) * P, osl],
                                      vs[:])
                else:
                    nc.vector.tensor_copy(vf[:, tch, osl], ps_v[:])

        if tp2:
            nc.gpsimd.collective_compute(
                "AllGather",
                mybir.AluOpType.bypass,
                replica_groups=[[2 * i, 2 * i + 1] for i in range(n_dev // 2)],
                ins=[vhalf_in.opt()],
                outs=[vhalf_out.opt()],
            )
            for half in range(2):
                for tch in range(tc_n):
                    nc.sync.dma_start(
                        vf[:, tch, half * hv:(half + 1) * hv],
                        vhalf_out[half, tch * P:(tch + 1) * P, :])

        # ------------- Phase Q: qm^T[a, s] = M^T hidden^T -------------------
        qmT = big.tile([P, hc, s_rows], bf, tag="C", name="qmT")
        for sh in range(s512):
            hid = big.tile([P, hc, 512], bf, tag="D", name="hid")
            nc.sync.dma_start(hid[:], d_h[:, :, sh * 512:(sh + 1) * 512])
            for oc in range(hc):
                m_oc = pws.tile([P, hc, P], bf, tag="W1", name="m_oc")
                nc.sync.dma_start(m_oc[:], d_m[:, :, oc * P:(oc + 1) * P])
                ps_q = pp.tile([P, 512], f32, tag="PS", name="ps_q")
                for ch in range(hc):
                    nc.tensor.matmul(ps_q[:], m_oc[:, ch, :], hid[:, ch, :],
                                     start=(ch == 0), stop=(ch == hc - 1))
                nc.vector.tensor_copy(qmT[:, oc, sh * 512:(sh + 1) * 512],
                                      ps_q[:])

        # ------------- Phase S: scores^T, exp, sums -------------------------
        eT = big.tile([P, tc_n, s_rows], bf, tag="D", name="eT")
        ones_t = psm.tile([P, 1], bf, tag="o1", name="ones_t")
        nc.vector.memset(ones_t[:], 1.0)
        pse = []
        for st in range(s512):
            t_ = ppe.tile([P, 512], f32, tag="PSE", name="ps_sum")
            pse.append(t_)
        for tch in range(tc_n):
            ps_sc = []
            for st in range(s512):
                ps_x = pp.tile([P, 512], f32, tag="PS", name="ps_sc")
                ps_sc.append(ps_x)
                for ch in range(hc):
                    nc.tensor.matmul(ps_x[:], a0[:, ch, tch * P:(tch + 1) * P],
                                     qmT[:, ch, st * 512:(st + 1) * 512],
                                     start=(ch == 0), stop=(ch == hc - 1))
            for st in range(s512):
                nc.scalar.activation(eT[:, tch, st * 512:(st + 1) * 512],
                                     ps_sc[st][:], AF.Exp, bias=zbias[:],
                                     scale=SCALE)
            for st in range(s512):
                nc.tensor.matmul(pse[st][0:1, :], ones_t[:],
                                 eT[:, tch, st * 512:(st + 1) * 512],
                                 start=(tch == 0), stop=(tch == tc_n - 1))

        recip = psm.tile([1, s_rows], f32, tag="rc", name="recip")
        for st in range(s512):
            nc.vector.reciprocal(recip[:, st * 512:(st + 1) * 512],
                                 pse[st][0:1, :])
        # Broadcast partition 0 to all partitions via a K=1 fp32 matmul
        ones_b = psm.tile([1, P], f32, tag="ob1", name="ones_b")
        nc.vector.memset(ones_b[:], 1.0)
        bcast = psm.tile([P, s_rows], f32, tag="bc", name="bcast")
        for st in range(s512):
            ps_b = pp.tile([P, 512], f32, tag="PS", name="ps_b")
            nc.tensor.matmul(ps_b[:], ones_b[:],
                             recip[:, st * 512:(st + 1) * 512])
            nc.vector.tensor_copy(bcast[:, st * 512:(st + 1) * 512], ps_b[:])

        # ------------- Phase C: ctx^T[h, s] (normalized) --------------------
        cT = big.tile([P, hc, s_rows], bf, tag="A", name="cT")
        for ch in range(hc):
            for st in range(s512):
                ps_c = pp.tile([P, 512], f32, tag="PS", name="ps_c")
                for tch in range(tc_n):
                    nc.tensor.matmul(ps_c[:], vf[:, tch, ch * P:(ch + 1) * P],
                                     eT[:, tch, st * 512:(st + 1) * 512],
                                     start=(tch == 0), stop=(tch == tc_n - 1))
                nc.vector.tensor_tensor(cT[:, ch, st * 512:(st + 1) * 512],
                                        ps_c[:],
                                        bcast[:, st * 512:(st + 1) * 512],
                                        AL.mult)

        # ------------- Phase O: out[s, o] = ctx Wo^T ------------------------
        for ot in range(n512):
            osl = slice(ot * 512, (ot + 1) * 512)
            wo_ot = pw.tile([P, hc, 512], bf, tag="W4", name="wo_ot")
            nc.sync.dma_start(wo_ot[:], d_wo[:, :, osl])
            for sc in range(sc_n):
                ps_o = pp.tile([P, 512], f32, tag="PS", name="ps_o")
                for ch in range(hc):
                    nc.tensor.matmul(ps_o[:], cT[:, ch, sc * P:(sc + 1) * P],
                                     wo_ot[:, ch, :],
                                     start=(ch == 0), stop=(ch == hc - 1))
                ob = pgs.tile([P, 512], f32, tag="OB", name="ob")
                nc.vector.tensor_copy(ob[:], ps_o[:])
                nc.sync.dma_start(d_out[sc * P:(sc + 1) * P, osl], ob[:])

    nc.compile()
    return nc


def _to_dev_layout(x_t, rows):
    """[rows, n] fp32 -> [128, rows//128, n] bf16 contiguous."""
    rc = rows // P
    return np.ascontiguousarray(
        x_t.reshape(rc, P, -1).transpose(1, 0, 2).astype(bf16))


def _gate_prep(trip, rid, ipad, apad):
    """Host-side gate folding for one batch.

    trip: [T, 3, h] fp32; rid: [T] ids.
    Returns adv_lin [T,h], si [ipad,h], da [apad,h], Pi [T,ipad], Pa [T,apad].
    """
    t_n = trip.shape[0]
    m_and = rid == 0
    m_or = rid == 1
    m_not = rid == 2
    m_impl = rid == 3
    m_xor = rid == 4
    c0 = (rid >= 5).astype(np.float32)
    ca = m_and.astype(np.float32) - m_xor.astype(np.float32)
    cb = m_or.astype(np.float32) + m_xor.astype(np.float32)
    c1 = -(m_not.astype(np.float32))
    ci = m_impl.astype(np.float32)
    k_s = (ca + cb + c1) / 2
    k_d = (c1 - ci) / 2
    k_as = ci / 2
    k_ad = (cb - ca) / 2

    a0 = trip[:, 0]
    asum = trip[:, 1] + trip[:, 2]
    adif = trip[:, 1] - trip[:, 2]
    adv_lin = c0[:, None] * a0 + k_s[:, None] * asum + k_d[:, None] * adif

    h = trip.shape[2]
    impl_idx = np.where(m_impl)[0]
    aox_idx = np.where(m_and | m_or | m_xor)[0]
    assert len(impl_idx) <= ipad, f"impl rows {len(impl_idx)} > pad {ipad}"
    assert len(aox_idx) <= apad, f"aox rows {len(aox_idx)} > pad {apad}"
    si = np.zeros((ipad, h), np.float32)
    si[:len(impl_idx)] = k_as[impl_idx, None] * asum[impl_idx]
    da = np.zeros((apad, h), np.float32)
    da[:len(aox_idx)] = np.abs(k_ad[aox_idx, None]) * adif[aox_idx]
    Pi = np.zeros((t_n, ipad), np.float32)
    Pi[impl_idx, np.arange(len(impl_idx))] = 1.0
    Pa = np.zeros((t_n, apad), np.float32)
    Pa[aox_idx, np.arange(len(aox_idx))] = np.sign(k_ad[aox_idx])
    return adv_lin, si, da, Pi, Pa


def kernel(hidden_states, advisor_states, advisor_ids, Wq, Wk, Wv, Wo):
    from concourse.bass_utils import run_bass_kernel_spmd

    hs = np.asarray(hidden_states, dtype=np.float32)     # [4, 2048, 2048]
    adv = np.asarray(advisor_states, dtype=np.float32)   # [4, 3072, 2048]
    ids = np.asarray(advisor_ids)                        # [4, 3072]

    # Size the compact-gate pads to the data (multiple of 128, with the
    # compiled defaults as minimum). Rebuild only if the data needs more.
    rid_all = ids.reshape(B, T, 3)[:, :, 0]
    need_i = int(max((rid_all[b] == 3).sum() for b in range(B)))
    need_a = int(max(((rid_all[b] == 0) | (rid_all[b] == 1)
                      | (rid_all[b] == 4)).sum() for b in range(B)))
    ipad = max(IPAD, -(-need_i // P) * P)
    apad = max(APAD, -(-need_a // P) * P)

    global _compiled_nc
    if _compiled_nc is None or _compiled_nc[0] != (ipad, apad, TP2):
        _compiled_nc = ((ipad, apad, TP2),
                        _build_nc(ipad=ipad, apad=apad, tp2=TP2))
    nc = _compiled_nc[1]
    Wq = np.asarray(Wq, dtype=np.float32)
    Wk = np.asarray(Wk, dtype=np.float32)
    Wv = np.asarray(Wv, dtype=np.float32)
    Wo = np.asarray(Wo, dtype=np.float32)

    M = Wq.T @ Wk                                        # folds K projection
    w_dev = {
        "m": _to_dev_layout(np.ascontiguousarray(M), H),
        "wo": _to_dev_layout(np.ascontiguousarray(Wo.T), H),
    }
    WvT = Wv.T
    if TP2:
        wv_half = [
            _to_dev_layout(np.ascontiguousarray(WvT[:, :H // 2]), H),
            _to_dev_layout(np.ascontiguousarray(WvT[:, H // 2:]), H),
        ]
    else:
        wv_full = _to_dev_layout(np.ascontiguousarray(WvT), H)

    per_batch = []
    for b in range(B):
        trip = adv[b].reshape(T, 3, H)
        rid = ids[b].reshape(T, 3)[:, 0]
        adv_lin, si, da, Pi, Pa = _gate_prep(trip, rid, ipad, apad)
        per_batch.append({
            "a0": _to_dev_layout(np.ascontiguousarray(trip[:, 0].T), H),
            "alT": _to_dev_layout(np.ascontiguousarray(adv_lin.T), H),
            "siT": _to_dev_layout(np.ascontiguousarray(si.T), H),
            "daT": _to_dev_layout(np.ascontiguousarray(da.T), H),
            "piT": _to_dev_layout(np.ascontiguousarray(Pi.T), ipad),
            "paT": _to_dev_layout(np.ascontiguousarray(Pa.T), apad),
        })

    in_maps = []
    for c in range(NCORES):
        b, sh = c // 2, c % 2
        hT = np.ascontiguousarray(hs[b, sh * S:(sh + 1) * S, :].T)
        m = {
            "hT": _to_dev_layout(hT, H),
            "wv": wv_half[sh] if TP2 else wv_full,
            **per_batch[b],
            **w_dev,
        }
        in_maps.append(m)

    res = run_bass_kernel_spmd(nc, in_maps, core_ids=list(range(NCORES)))
    kernel._last_results = res

    out = np.empty((B, 2 * S, H), dtype=np.float32)
    for c in range(NCORES):
        b, sh = c // 2, c % 2
        out[b, sh * S:(sh + 1) * S, :] = res.results[c]["out"]
    return out
